# revision 1
# baseline (speedup 1.0000x reference)
"""Autoformer autocorrelation block on 8 trn2 NeuronCores.

Math: the reference computes corr = irfft(rfft(q)*conj(rfft(k))) along L and
then takes mean over (H, L-lags).  Sum over all circular lags of a circular
cross-correlation factorizes: sum_d corr[d] = (sum_t q[t]) * (sum_s k[s]).
So mean_value[b,e] = (1/(H*L)) * sum_h colsum_q[b,h,e] * colsum_k[b,h,e]
— no FFT needed, and only column sums of the projected q/k, which equal
(colsum(queries) @ Wq + L*bq).  Top-k indices (k=7, over E=64) become roll
shifts s in [0,64); the weighted roll-aggregation is a 7-tap circular filter
along L, expressed as two banded 128x128 matmuls per 128-row block.

Device work (per core, data-parallel over batch B=8):
  phase 1: column sums of queries[b], keys[b]              (16 MB DMA, tiny PE)
  phase 2: v = values@Wv ; aggT = band-matmul ; out = agg@Wo + bias
Host glue: [8,1024]x[1024,1024] sum-projections, top-7 of 64, softmax of 7,
building two 128x128 band matrices — all O(MB) scalar work.
"""

import os

import numpy as np

import concourse.bass as bass
import concourse.tile as tile
from concourse import bacc
from concourse import mybir
from concourse.bass_utils import run_bass_kernel_spmd

LAST_EXEC_NS = []
LAST_WALL_NS = []


def _run(nc, in_maps):
    import time
    trace = bool(os.environ.get("KTRACE"))
    t0 = time.time()
    try:
        res = run_bass_kernel_spmd(nc, in_maps,
                                   core_ids=list(range(len(in_maps))),
                                   trace=trace)
    except ModuleNotFoundError:
        res = run_bass_kernel_spmd(nc, in_maps,
                                   core_ids=list(range(len(in_maps))),
                                   trace=False)
    LAST_WALL_NS.append(int((time.time() - t0) * 1e9))
    if res.exec_time_ns is not None:
        LAST_EXEC_NS.append(res.exec_time_ns)
    return res.results

B, L, D, H, E, TOPK = 8, 2048, 1024, 16, 64, 7
P = 128
NT = L // P   # 16 row blocks along L
ND = D // P   # 8 chunks along D
F32 = mybir.dt.float32
BF16 = mybir.dt.bfloat16

_NC_CACHE = {}


def build_phase1():
    nc = bacc.Bacc()
    q = nc.declare_dram_parameter("q", [L, D], F32, isOutput=False)
    k = nc.declare_dram_parameter("k", [L, D], F32, isOutput=False)
    out = nc.declare_dram_parameter("out", [2, D], F32, isOutput=True)
    with tile.TileContext(nc) as tc:
        with (
            tc.tile_pool(name="io", bufs=2) as io,
            tc.tile_pool(name="ones", bufs=1) as onesp,
            tc.tile_pool(name="ps", bufs=2, space="PSUM") as psp,
            tc.tile_pool(name="res", bufs=2) as resp,
        ):
            ones = onesp.tile([P, 1], F32)
            nc.vector.memset(ones[:], 1.0)
            warm = psp.tile([1, 1], F32, tag="warm")
            nc.tensor.matmul(warm[:], ones[:], ones[:], start=True, stop=True)
            for idx, src in enumerate((q, k)):
                big = io.tile([P, NT, D], F32)
                nc.sync.dma_start(
                    big[:], src.rearrange("(t p) d -> p t d", p=P))
                ps = psp.tile([1, D], F32)
                res = resp.tile([1, D], F32)
                for n in range(2):
                    sl = slice(n * 512, (n + 1) * 512)
                    for t in range(NT):
                        nc.tensor.matmul(ps[:, sl], ones[:], big[:, t, sl],
                                         start=(t == 0), stop=(t == NT - 1))
                nc.vector.tensor_copy(res[:], ps[:])
                nc.sync.dma_start(out[idx:idx + 1, :], res[:])
    nc.compile()
    return nc


def build_phase2():
    nc = bacc.Bacc()
    vT = nc.declare_dram_parameter("vT", [D, L], F32, isOutput=False)
    Wv = nc.declare_dram_parameter("Wv", [D, D], F32, isOutput=False)
    Wo = nc.declare_dram_parameter("Wo", [D, D], F32, isOutput=False)
    SaT = nc.declare_dram_parameter("SaT", [P, P], F32, isOutput=False)
    SbT = nc.declare_dram_parameter("SbT", [P, P], F32, isOutput=False)
    bo2 = nc.declare_dram_parameter("bo2", [D, 1], F32, isOutput=False)
    outT = nc.declare_dram_parameter("out", [D, L], F32, isOutput=True)

    with tile.TileContext(nc) as tc:
        with (
            tc.tile_pool(name="stage", bufs=3) as stp,
            tc.tile_pool(name="vTbf", bufs=1) as vtp,
            tc.tile_pool(name="wbf", bufs=1) as wbp,
            tc.tile_pool(name="sbf", bufs=1) as sbp,
            tc.tile_pool(name="vbf", bufs=1) as vp,
            tc.tile_pool(name="aggT", bufs=1) as agp,
            tc.tile_pool(name="outs", bufs=2) as otp,
            tc.tile_pool(name="psv", bufs=3, space="PSUM") as psv,
            tc.tile_pool(name="psb", bufs=2, space="PSUM") as psb,
            tc.tile_pool(name="pso", bufs=3, space="PSUM") as pso,
        ):
            # --- load + cast inputs to bf16 ---
            vT_bf = []
            for c in range(ND):
                st = stp.tile([P, L], F32, tag="stage")
                nc.sync.dma_start(st[:], vT[c * P:(c + 1) * P, :])
                t = vtp.tile([P, L], BF16, tag=f"vT{c}", name=f"vTbf{c}")
                nc.vector.tensor_copy(t[:], st[:])
                vT_bf.append(t)
            Wv_bf, Wo_bf = [], []
            for w_dram, lst in ((Wv, Wv_bf), (Wo, Wo_bf)):
                for c in range(ND):
                    st = stp.tile([P, D], F32, tag="stage")
                    nc.sync.dma_start(st[:], w_dram[c * P:(c + 1) * P, :])
                    t = wbp.tile([P, D], BF16, tag=f"w{w_dram.name}{c}", name=f"wbf_{w_dram.name}{c}")
                    nc.vector.tensor_copy(t[:], st[:])
                    lst.append(t)
            Sa_bf = sbp.tile([P, P], BF16)
            Sb_bf = sbp.tile([P, P], BF16)
            for s_dram, s_t in ((SaT, Sa_bf), (SbT, Sb_bf)):
                st = stp.tile([P, P], F32, tag="sstage")
                nc.sync.dma_start(st[:], s_dram[:, :])
                nc.vector.tensor_copy(s_t[:], st[:])
            bias = sbp.tile([P, ND], F32)
            nc.sync.dma_start(
                bias[:], bo2.rearrange("(c p) one -> p (c one)", p=P))

            # --- v projection: v[m] [t=128, d=1024], bf16 ---
            v_bf = [vp.tile([P, D], BF16, tag=f"v{i}", name=f"v{i}") for i in range(NT)]
            for m in range(NT):
                for n in range(2):
                    sl = slice(n * 512, (n + 1) * 512)
                    ps = psv.tile([P, 512], F32)
                    for kc in range(ND):
                        nc.tensor.matmul(
                            ps[:],
                            vT_bf[kc][:, m * P:(m + 1) * P],
                            Wv_bf[kc][:, sl],
                            start=(kc == 0), stop=(kc == ND - 1))
                    nc.vector.tensor_copy(v_bf[m][:, sl], ps[:])

            # --- banded circular aggregation: aggT[dc] [d=128, t=2048] ---
            agg_bf = [agp.tile([P, L], BF16, tag=f"agg{i}", name=f"agg{i}") for i in range(ND)]
            for dc in range(ND):
                dsl = slice(dc * P, (dc + 1) * P)
                for mg in range(4):
                    ps = psb.tile([P, 512], F32)
                    for j in range(4):
                        m = mg * 4 + j
                        osl = slice(j * P, (j + 1) * P)
                        nc.tensor.matmul(ps[:, osl], v_bf[m][:, dsl],
                                         Sa_bf[:], start=True, stop=False)
                        nc.tensor.matmul(ps[:, osl],
                                         v_bf[(m + 1) % NT][:, dsl],
                                         Sb_bf[:], start=False, stop=True)
                    nc.vector.tensor_copy(
                        agg_bf[dc][:, mg * 512:(mg + 1) * 512], ps[:])

            # --- output projection + bias: outT[d2=128, t=2048] f32 ---
            for dc2 in range(ND):
                ot = otp.tile([P, L], F32)
                for n4 in range(4):
                    sl = slice(n4 * 512, (n4 + 1) * 512)
                    ps = pso.tile([P, 512], F32)
                    for kc in range(ND):
                        nc.tensor.matmul(
                            ps[:],
                            Wo_bf[kc][:, dc2 * P:(dc2 + 1) * P],
                            agg_bf[kc][:, sl],
                            start=(kc == 0), stop=(kc == ND - 1))
                    nc.vector.tensor_scalar_add(
                        ot[:, sl], ps[:], bias[:, dc2:dc2 + 1])
                nc.sync.dma_start(outT[dc2 * P:(dc2 + 1) * P, :], ot[:])
    nc.compile()
    return nc


def _softmax(x, axis=-1):
    m = x.max(axis=axis, keepdims=True)
    e = np.exp(x - m)
    return e / e.sum(axis=axis, keepdims=True)


def host_glue(csq, csk, Wq, bq, Wk, bk, bv, Wo, bo):
    """From per-batch column sums of queries/keys -> band matrices + bias."""
    qs = csq.astype(np.float64) @ Wq.astype(np.float64) + L * bq
    ks = csk.astype(np.float64) @ Wk.astype(np.float64) + L * bk
    mv = (qs.reshape(B, H, E) * ks.reshape(B, H, E)).sum(1) / (H * L)  # [B,E]
    idx = np.argsort(-mv.mean(0))[:TOPK]
    w = _softmax(mv[:, idx], axis=-1)  # [B, TOPK]
    SaT = np.zeros((B, P, P), np.float32)
    SbT = np.zeros((B, P, P), np.float32)
    for b in range(B):
        for i, s in enumerate(idx):
            s = int(s)
            SaT[b] += np.eye(P, k=-s, dtype=np.float32) * w[b, i]
            if s > 0:
                SbT[b] += np.eye(P, k=P - s, dtype=np.float32) * w[b, i]
    bo2 = (bv.astype(np.float64) @ Wo.astype(np.float64) + bo)
    return SaT, SbT, bo2.astype(np.float32).reshape(D, 1)


def kernel(**inputs):
    f = lambda k: np.ascontiguousarray(np.asarray(inputs[k], dtype=np.float32))
    queries, keys, values = f("queries"), f("keys"), f("values")
    Wq, bq, Wk, bk = f("Wq"), f("bq"), f("Wk"), f("bk")
    Wv, bv, Wo, bo = f("Wv"), f("bv"), f("Wo"), f("bo")

    if "p1" not in _NC_CACHE:
        _NC_CACHE["p1"] = build_phase1()
    nc1 = _NC_CACHE["p1"]
    in1 = [{"q": np.ascontiguousarray(queries[b]),
            "k": np.ascontiguousarray(keys[b])} for b in range(B)]
    r1 = _run(nc1, in1)
    csq = np.stack([r1[b]["out"][0] for b in range(B)])
    csk = np.stack([r1[b]["out"][1] for b in range(B)])

    SaT, SbT, bo2 = host_glue(csq, csk, Wq, bq, Wk, bk, bv, Wo, bo)

    if "p2" not in _NC_CACHE:
        _NC_CACHE["p2"] = build_phase2()
    nc2 = _NC_CACHE["p2"]
    # fold bv into the kernel bias: out = (S@(values@Wv))@Wo + (bv@Wo + bo)
    # (valid because each row of S sums to 1 — softmax weights)
    in2 = [{"vT": np.ascontiguousarray(values[b].T),
            "Wv": Wv, "Wo": Wo,
            "SaT": SaT[b], "SbT": SbT[b], "bo2": bo2} for b in range(B)]
    r2 = _run(nc2, in2)
    out = np.stack([np.ascontiguousarray(r2[b]["out"].T) for b in range(B)])
    return out.astype(np.float32)



# revision 2
# speedup vs baseline: 6.2198x; 6.2198x over previous
"""Autoformer autocorrelation block on 8 trn2 NeuronCores — single launch.

Math: the reference computes corr = irfft(rfft(q)*conj(rfft(k))) along L and
takes mean over (H, L-lags).  Sum over all circular lags of a circular
cross-correlation factorizes: sum_d corr[d] = (sum_t q[t]) * (sum_s k[s]),
so mean_value[b,e] = (1/(H*L)) * sum_h colsum_q[b,h,e] * colsum_k[b,h,e]
— no FFT needed, only column sums of the projected q/k, which equal
(colsum(queries) @ Wq + L*bq).  Those column sums are O(MB) host work.

Top-k indices (k=7, over E=64) become roll shifts s in [0,64); the weighted
roll-aggregation is a 7-tap circular filter along L.  The filter S acts on
the L axis while Wv/Wo act on the channel axis, so they commute:
  out = S@(values@Wv + bv)@Wo + bo = (S@values)@(Wv@Wo) + (bv@Wo + bo)
Host folds W = Wv@Wo and the bias; the device (one core per batch element)
does the banded circular filter (two 128x128 matmuls per 128-row block)
followed by ONE 2048x1024x1024 GEMM in bf16, bias fused into the PSUM->SBUF
copy.  No transposes on either side: values ships as [L, D] bf16, output
returns as [L, D] f32.
"""

import os

import ml_dtypes
import numpy as np

import concourse.bass as bass  # noqa: F401
import concourse.tile as tile
from concourse import bacc
from concourse import mybir
from concourse.bass_utils import run_bass_kernel_spmd

LAST_EXEC_NS = []
LAST_WALL_NS = []

B, L, D, H, E, TOPK = 8, 2048, 1024, 16, 64, 7
P = 128
NT = L // P   # 16 row blocks along L
ND = D // P   # 8 chunks along D
F32 = mybir.dt.float32
BF16 = mybir.dt.bfloat16
BF16_NP = ml_dtypes.bfloat16

_NC_CACHE = {}


def _run(nc, in_maps):
    import time
    trace = bool(os.environ.get("KTRACE"))
    t0 = time.time()
    try:
        res = run_bass_kernel_spmd(nc, in_maps,
                                   core_ids=list(range(len(in_maps))),
                                   trace=trace)
    except ModuleNotFoundError:
        res = run_bass_kernel_spmd(nc, in_maps,
                                   core_ids=list(range(len(in_maps))),
                                   trace=False)
    LAST_WALL_NS.append(int((time.time() - t0) * 1e9))
    if res.exec_time_ns is not None:
        LAST_EXEC_NS.append(res.exec_time_ns)
    return res.results


def build_kernel():
    nc = bacc.Bacc()
    v_d = nc.declare_dram_parameter("v", [L, D], BF16, isOutput=False)
    W_d = nc.declare_dram_parameter("W", [D, D], BF16, isOutput=False)
    Sa_d = nc.declare_dram_parameter("Sa", [P, P], BF16, isOutput=False)
    Sb_d = nc.declare_dram_parameter("Sb", [P, P], BF16, isOutput=False)
    bb_d = nc.declare_dram_parameter("bb", [P, D], F32, isOutput=False)
    out_d = nc.declare_dram_parameter("out", [L, D], F32, isOutput=True)

    with tile.TileContext(nc) as tc:
        with (
            tc.tile_pool(name="vbf", bufs=1) as vp,
            tc.tile_pool(name="wbf", bufs=1) as wp,
            tc.tile_pool(name="sbf", bufs=1) as sp,
            tc.tile_pool(name="agg", bufs=1) as agp,
            tc.tile_pool(name="outs", bufs=3) as otp,
            tc.tile_pool(name="psw", bufs=1, space="PSUM") as psw,
            tc.tile_pool(name="psb", bufs=2, space="PSUM") as psb,
            tc.tile_pool(name="pso", bufs=4, space="PSUM") as pso,
        ):
            Sa = sp.tile([P, P], BF16, name="Sa")
            Sb = sp.tile([P, P], BF16, name="Sb")
            nc.sync.dma_start(Sa[:], Sa_d[:, :])
            nc.sync.dma_start(Sb[:], Sb_d[:, :])

            # PE warmup (HAM clock ramp) overlapping the input DMAs
            ones = sp.tile([P, 1], F32, name="ones")
            nc.vector.memset(ones[:], 1.0)
            warm = psw.tile([1, 1], F32, tag="warm")
            nc.tensor.matmul(warm[:], ones[:], ones[:], start=True, stop=True)

            v_t = []
            for m in range(NT):
                t = vp.tile([P, D], BF16, tag=f"v{m}", name=f"v{m}")
                nc.sync.dma_start(t[:], v_d[m * P:(m + 1) * P, :])
                v_t.append(t)
            W_t = []
            for c in range(ND):
                t = wp.tile([P, D], BF16, tag=f"W{c}", name=f"W{c}")
                nc.sync.dma_start(t[:], W_d[c * P:(c + 1) * P, :])
                W_t.append(t)
            bias = sp.tile([P, D], F32, name="bias")
            nc.sync.dma_start(bias[:], bb_d[:, :])

            # banded circular aggregation: aggT[dc] = [d=128, t=2048] bf16
            # aggT[d, t] = sum_tin v[tin, d] * (Sa|Sb)[tin, t]
            agg_t = [agp.tile([P, L], BF16, tag=f"agg{c}", name=f"agg{c}")
                     for c in range(ND)]
            for mg in range(NT // 4):
                for dc in range(ND):
                    ps = psb.tile([P, 512], F32)
                    dsl = slice(dc * P, (dc + 1) * P)
                    for j in range(4):
                        m = mg * 4 + j
                        osl = slice(j * P, (j + 1) * P)
                        nc.tensor.matmul(ps[:, osl], v_t[m][:, dsl], Sa[:],
                                         start=True, stop=False)
                        nc.tensor.matmul(ps[:, osl], v_t[(m + 1) % NT][:, dsl],
                                         Sb[:], start=False, stop=True)
                    nc.vector.tensor_copy(
                        agg_t[dc][:, mg * 512:(mg + 1) * 512], ps[:])

            # out[m] = agg[:, m].T @ W + bias   -> [t=128, n=1024] f32
            for m in range(NT):
                ot = otp.tile([P, D], F32)
                for nh in range(2):
                    sl = slice(nh * 512, (nh + 1) * 512)
                    ps = pso.tile([P, 512], F32)
                    for kc in range(ND):
                        nc.tensor.matmul(
                            ps[:],
                            agg_t[kc][:, m * P:(m + 1) * P],
                            W_t[kc][:, sl],
                            start=(kc == 0), stop=(kc == ND - 1))
                    nc.vector.scalar_tensor_tensor(
                        ot[:, sl], ps[:], 1.0, bias[:, sl],
                        op0=mybir.AluOpType.mult, op1=mybir.AluOpType.add)
                nc.sync.dma_start(out_d[m * P:(m + 1) * P, :], ot[:])
    nc.compile()
    return nc


def _softmax(x, axis=-1):
    m = x.max(axis=axis, keepdims=True)
    e = np.exp(x - m)
    return e / e.sum(axis=axis, keepdims=True)


def host_prep(queries, keys, Wq, bq, Wk, bk, Wv, bv, Wo, bo):
    """Column sums -> top-k shifts + softmax -> band matrices; fold Wv@Wo."""
    csq = queries.sum(axis=1, dtype=np.float64)          # [B, D]
    csk = keys.sum(axis=1, dtype=np.float64)             # [B, D]
    qs = csq @ Wq.astype(np.float64) + L * bq.astype(np.float64)
    ks = csk @ Wk.astype(np.float64) + L * bk.astype(np.float64)
    mv = (qs.reshape(B, H, E) * ks.reshape(B, H, E)).sum(1) / (H * L)  # [B,E]
    idx = np.argsort(-mv.mean(0), kind="stable")[:TOPK]
    w = _softmax(mv[:, idx], axis=-1)                    # [B, TOPK]
    SaT = np.zeros((B, P, P), np.float32)
    SbT = np.zeros((B, P, P), np.float32)
    for b in range(B):
        for i, s in enumerate(idx):
            s = int(s)
            SaT[b] += np.eye(P, k=-s, dtype=np.float32) * np.float32(w[b, i])
            if s > 0:
                SbT[b] += np.eye(P, k=P - s, dtype=np.float32) * np.float32(w[b, i])
    Wf = (Wv.astype(np.float64) @ Wo.astype(np.float64)).astype(np.float32)
    bias = (bv.astype(np.float64) @ Wo.astype(np.float64) + bo).astype(np.float32)
    return SaT, SbT, Wf, bias


def kernel(**inputs):
    f = lambda k: np.ascontiguousarray(np.asarray(inputs[k], dtype=np.float32))
    queries, keys, values = f("queries"), f("keys"), f("values")
    Wq, bq, Wk, bk = f("Wq"), f("bq"), f("Wk"), f("bk")
    Wv, bv, Wo, bo = f("Wv"), f("bv"), f("Wo"), f("bo")

    SaT, SbT, Wf, bias = host_prep(queries, keys, Wq, bq, Wk, bk,
                                   Wv, bv, Wo, bo)
    W_bf = Wf.astype(BF16_NP)
    Sa_bf = SaT.astype(BF16_NP)
    Sb_bf = SbT.astype(BF16_NP)
    bias_bc = np.ascontiguousarray(
        np.broadcast_to(bias, (P, D)).astype(np.float32))

    if "k1" not in _NC_CACHE:
        _NC_CACHE["k1"] = build_kernel()
    nc = _NC_CACHE["k1"]
    in_maps = [{"v": values[b].astype(BF16_NP),
                "W": W_bf,
                "Sa": Sa_bf[b], "Sb": Sb_bf[b],
                "bb": bias_bc} for b in range(B)]
    r = _run(nc, in_maps)
    out = np.stack([r[b]["out"] for b in range(B)])
    return out.astype(np.float32)


# revision 8
# speedup vs baseline: 7.6764x; 1.2342x over previous
"""Autoformer autocorrelation block on 8 trn2 NeuronCores — single launch.

Math: the reference computes corr = irfft(rfft(q)*conj(rfft(k))) along L and
takes mean over (H, L-lags).  Sum over all circular lags of a circular
cross-correlation factorizes: sum_d corr[d] = (sum_t q[t]) * (sum_s k[s]),
so mean_value[b,e] = (1/(H*L)) * sum_h colsum_q[b,h,e] * colsum_k[b,h,e]
— no FFT needed, only column sums of the projected q/k, which equal
(colsum(queries) @ Wq + L*bq).  Those column sums are O(MB) host work.

Top-k indices (k=7, over E=64) become roll shifts s in [0,64); the weighted
roll-aggregation is a 7-tap circular filter along L.  The filter S acts on
the L axis while Wv/Wo act on the channel axis, so they commute:
  out = S@(values@Wv + bv)@Wo + bo = (S@values)@(Wv@Wo) + (bv@Wo + bo)
Host folds W = Wv@Wo and the bias; the device (one core per batch element)
does the banded circular filter (two 128x128 matmuls per 128-row block)
followed by ONE 2048x1024x1024 GEMM in bf16, bias fused into the PSUM->SBUF
copy.  No transposes on either side: values ships as [L, D] bf16, output
returns as [L, D] f32.
"""

import os

import ml_dtypes
import numpy as np

import concourse.bass as bass  # noqa: F401
import concourse.tile as tile
from concourse import bacc
from concourse import mybir
from concourse.bass_utils import run_bass_kernel_spmd

LAST_EXEC_NS = []
LAST_WALL_NS = []

B, L, D, H, E, TOPK = 8, 2048, 1024, 16, 64, 7
P = 128
NT = L // P   # 16 row blocks along L
ND = D // P   # 8 chunks along D
F32 = mybir.dt.float32
BF16 = mybir.dt.bfloat16
BF16_NP = ml_dtypes.bfloat16

_NC_CACHE = {}


def _run_fallback(nc, in_maps):
    """Stock runner (used if the cached fast path breaks)."""
    res = run_bass_kernel_spmd(nc, in_maps,
                               core_ids=list(range(len(in_maps))),
                               trace=False)
    if res.exec_time_ns is not None:
        LAST_EXEC_NS.append(res.exec_time_ns)
    return [r["out"] for r in res.results]


def _get_runner(nc):
    """Cached jit runner: replicated weights, device-resident zero output
    buffers (no per-call host->device upload of them), no donation so the
    cached zeros stay valid, bf16 output fetch."""
    import jax
    import jax.numpy as jnp
    from jax.sharding import Mesh, PartitionSpec, NamedSharding
    from jax.experimental.shard_map import shard_map
    from concourse.bass2jax import (_bass_exec_p, install_neuronx_cc_hook,
                                    partition_id_tensor)
    install_neuronx_cc_hook()

    partition_name = (nc.partition_id_tensor.name
                      if nc.partition_id_tensor else None)
    in_names, out_names, out_avals = [], [], []
    for alloc in nc.m.functions[0].allocations:
        if not isinstance(alloc, mybir.MemoryLocationSet):
            continue
        name = alloc.memorylocations[0].name
        if alloc.kind == "ExternalInput":
            if name != partition_name:
                in_names.append(name)
        elif alloc.kind == "ExternalOutput":
            out_names.append(name)
            out_avals.append(jax.core.ShapedArray(
                tuple(alloc.tensor_shape), mybir.dt.np(alloc.dtype)))
    assert in_names == ["v", "W", "Sa", "Sb", "bb"], in_names
    in_names_all = in_names + out_names + (
        [partition_name] if partition_name else [])

    def _body(*args):
        operands = list(args)
        if partition_name is not None:
            operands.append(partition_id_tensor())
        outs = _bass_exec_p.bind(
            *operands,
            out_avals=tuple(out_avals),
            in_names=tuple(in_names_all),
            out_names=tuple(out_names),
            lowering_input_output_aliases=(),
            sim_require_finite=True,
            sim_require_nnan=True,
            nc=nc)
        return tuple(outs)

    devices = jax.devices()[:B]
    mesh = Mesh(np.asarray(devices), ("core",))
    SH = PartitionSpec("core")
    RE = PartitionSpec()
    # param order: v, W, Sa, Sb, bb, then zero output buffers
    in_specs = (SH, RE, SH, SH, RE) + (SH,) * len(out_names)
    out_specs = (SH,) * len(out_names)
    sharded = jax.jit(
        shard_map(_body, mesh=mesh, in_specs=in_specs,
                  out_specs=out_specs, check_rep=False),
        keep_unused=True)
    zero_sh = NamedSharding(mesh, SH)
    zeros = [
        jax.jit(lambda a=a: jnp.zeros((B * a.shape[0],) + a.shape[1:],
                                      a.dtype),
                out_shardings=zero_sh)()
        for a in out_avals
    ]
    for z in zeros:
        z.block_until_ready()
    return sharded, zeros


def _run(nc, v_cat, W_bf, Sa_cat, Sb_cat, bb):
    import time
    t0 = time.time()
    if os.environ.get("KFALLBACK"):
        in_maps = [{"v": v_cat[b * L:(b + 1) * L], "W": W_bf,
                    "Sa": Sa_cat[b * P:(b + 1) * P],
                    "Sb": Sb_cat[b * P:(b + 1) * P],
                    "bb": bb} for b in range(B)]
        outs = _run_fallback(nc, in_maps)
        out = np.stack(outs).astype(np.float32)
    else:
        if "runner" not in _NC_CACHE:
            _NC_CACHE["runner"] = _get_runner(nc)
        sharded, zeros = _NC_CACHE["runner"]
        outs = sharded(v_cat, W_bf, Sa_cat, Sb_cat, bb, *zeros)
        out = np.asarray(outs[0]).reshape(B, L, D).astype(np.float32)
    LAST_WALL_NS.append(int((time.time() - t0) * 1e9))
    return out


def build_kernel():
    nc = bacc.Bacc()
    v_d = nc.declare_dram_parameter("v", [L, D], BF16, isOutput=False)
    W_d = nc.declare_dram_parameter("W", [D, D], BF16, isOutput=False)
    Sa_d = nc.declare_dram_parameter("Sa", [P, P], BF16, isOutput=False)
    Sb_d = nc.declare_dram_parameter("Sb", [P, P], BF16, isOutput=False)
    bb_d = nc.declare_dram_parameter("bb", [P, D], F32, isOutput=False)
    out_d = nc.declare_dram_parameter("out", [L, D], BF16, isOutput=True)

    with tile.TileContext(nc) as tc:
        with (
            tc.tile_pool(name="vbf", bufs=1) as vp,
            tc.tile_pool(name="wbf", bufs=1) as wp,
            tc.tile_pool(name="sbf", bufs=1) as sp,
            tc.tile_pool(name="agg", bufs=1) as agp,
            tc.tile_pool(name="outs", bufs=3) as otp,
            tc.tile_pool(name="psw", bufs=1, space="PSUM") as psw,
            tc.tile_pool(name="psb", bufs=2, space="PSUM") as psb,
            tc.tile_pool(name="pso", bufs=4, space="PSUM") as pso,
        ):
            Sa = sp.tile([P, P], BF16, name="Sa")
            Sb = sp.tile([P, P], BF16, name="Sb")
            nc.sync.dma_start(Sa[:], Sa_d[:, :])
            nc.sync.dma_start(Sb[:], Sb_d[:, :])

            # PE warmup (HAM clock ramp) overlapping the input DMAs
            ones = sp.tile([P, 1], F32, name="ones")
            nc.vector.memset(ones[:], 1.0)
            warm = psw.tile([1, 1], F32, tag="warm")
            nc.tensor.matmul(warm[:], ones[:], ones[:], start=True, stop=True)

            v_t = []
            for m in range(NT):
                t = vp.tile([P, D], BF16, tag=f"v{m}", name=f"v{m}")
                nc.sync.dma_start(t[:], v_d[m * P:(m + 1) * P, :])
                v_t.append(t)
            W_t = []
            for c in range(ND):
                t = wp.tile([P, D], BF16, tag=f"W{c}", name=f"W{c}")
                nc.sync.dma_start(t[:], W_d[c * P:(c + 1) * P, :])
                W_t.append(t)
            bias = sp.tile([P, D], F32, name="bias")
            nc.sync.dma_start(bias[:], bb_d[:, :])

            # banded circular aggregation: aggT[dc] = [d=128, t=2048] bf16
            # aggT[d, t] = sum_tin v[tin, d] * (Sa|Sb)[tin, t]
            agg_t = [agp.tile([P, L], BF16, tag=f"agg{c}", name=f"agg{c}")
                     for c in range(ND)]
            for mg in range(NT // 4):
                for dc in range(ND):
                    ps = psb.tile([P, 512], F32)
                    dsl = slice(dc * P, (dc + 1) * P)
                    for j in range(4):
                        m = mg * 4 + j
                        osl = slice(j * P, (j + 1) * P)
                        nc.tensor.matmul(ps[:, osl], v_t[m][:, dsl], Sa[:],
                                         start=True, stop=False)
                        nc.tensor.matmul(ps[:, osl], v_t[(m + 1) % NT][:, dsl],
                                         Sb[:], start=False, stop=True)
                    nc.vector.tensor_copy(
                        agg_t[dc][:, mg * 512:(mg + 1) * 512], ps[:])

            # out[m] = agg[:, m].T @ W + bias   -> [t=128, n=1024] bf16
            for m in range(NT):
                ot = otp.tile([P, D], BF16)
                for nh in range(2):
                    sl = slice(nh * 512, (nh + 1) * 512)
                    ps = pso.tile([P, 512], F32)
                    for kc in range(ND):
                        nc.tensor.matmul(
                            ps[:],
                            agg_t[kc][:, m * P:(m + 1) * P],
                            W_t[kc][:, sl],
                            start=(kc == 0), stop=(kc == ND - 1))
                    nc.vector.scalar_tensor_tensor(
                        ot[:, sl], ps[:], 1.0, bias[:, sl],
                        op0=mybir.AluOpType.mult, op1=mybir.AluOpType.add)
                nc.sync.dma_start(out_d[m * P:(m + 1) * P, :], ot[:])
    nc.compile()
    return nc


def _softmax(x, axis=-1):
    m = x.max(axis=axis, keepdims=True)
    e = np.exp(x - m)
    return e / e.sum(axis=axis, keepdims=True)


def host_prep(queries, keys, Wq, bq, Wk, bk, Wv, bv, Wo, bo):
    """Column sums -> top-k shifts + softmax -> band matrices; fold Wv@Wo."""
    csq = queries.sum(axis=1, dtype=np.float64)          # [B, D]
    csk = keys.sum(axis=1, dtype=np.float64)             # [B, D]
    qs = csq @ Wq.astype(np.float64) + L * bq.astype(np.float64)
    ks = csk @ Wk.astype(np.float64) + L * bk.astype(np.float64)
    mv = (qs.reshape(B, H, E) * ks.reshape(B, H, E)).sum(1) / (H * L)  # [B,E]
    idx = np.argsort(-mv.mean(0), kind="stable")[:TOPK]
    w = _softmax(mv[:, idx], axis=-1)                    # [B, TOPK]
    SaT = np.zeros((B, P, P), np.float32)
    SbT = np.zeros((B, P, P), np.float32)
    for b in range(B):
        for i, s in enumerate(idx):
            s = int(s)
            SaT[b] += np.eye(P, k=-s, dtype=np.float32) * np.float32(w[b, i])
            if s > 0:
                SbT[b] += np.eye(P, k=P - s, dtype=np.float32) * np.float32(w[b, i])
    Wf = (Wv.astype(np.float64) @ Wo.astype(np.float64)).astype(np.float32)
    bias = (bv.astype(np.float64) @ Wo.astype(np.float64) + bo).astype(np.float32)
    return SaT, SbT, Wf, bias


def kernel(**inputs):
    f = lambda k: np.ascontiguousarray(np.asarray(inputs[k], dtype=np.float32))
    queries, keys, values = f("queries"), f("keys"), f("values")
    Wq, bq, Wk, bk = f("Wq"), f("bq"), f("Wk"), f("bk")
    Wv, bv, Wo, bo = f("Wv"), f("bv"), f("Wo"), f("bo")

    SaT, SbT, Wf, bias = host_prep(queries, keys, Wq, bq, Wk, bk,
                                   Wv, bv, Wo, bo)
    W_bf = Wf.astype(BF16_NP)
    Sa_cat = SaT.reshape(B * P, P).astype(BF16_NP)
    Sb_cat = SbT.reshape(B * P, P).astype(BF16_NP)
    bias_bc = np.ascontiguousarray(
        np.broadcast_to(bias, (P, D)).astype(np.float32))
    v_cat = values.reshape(B * L, D).astype(BF16_NP)

    if "k1" not in _NC_CACHE:
        _NC_CACHE["k1"] = build_kernel()
    nc = _NC_CACHE["k1"]
    return _run(nc, v_cat, W_bf, Sa_cat, Sb_cat, bias_bc)


# revision 17
# speedup vs baseline: 8.7048x; 1.1340x over previous
"""Autoformer autocorrelation block on 8 trn2 NeuronCores — single launch.

Math: the reference computes corr = irfft(rfft(q)*conj(rfft(k))) along L and
takes mean over (H, L-lags).  Sum over all circular lags of a circular
cross-correlation factorizes: sum_d corr[d] = (sum_t q[t]) * (sum_s k[s]),
so mean_value[b,e] = (1/(H*L)) * sum_h colsum_q[b,h,e] * colsum_k[b,h,e]
— no FFT needed, only column sums of the projected q/k, which equal
(colsum(queries) @ Wq + L*bq).  Those column sums are O(MB) host work.

Top-k indices (k=7, over E=64) become roll shifts s in [0,64); the weighted
roll-aggregation is a 7-tap circular filter along L.  The filter S acts on
the L axis while Wv/Wo act on the channel axis, so they commute:
  out = S@(values@Wv + bv)@Wo + bo = (S@values)@(Wv@Wo) + (bv@Wo + bo)
Host folds W = Wv@Wo and the bias; the device (one core per batch element)
does the banded circular filter (two 128x128 matmuls per 128-row block)
followed by ONE 2048x1024x1024 GEMM in bf16, bias fused into the PSUM->SBUF
copy.  No transposes on either side: values ships as [L, D] bf16, output
returns as [L, D] f32.
"""

import os

import ml_dtypes
import numpy as np

import concourse.bass as bass  # noqa: F401
import concourse.tile as tile
from concourse import bacc
from concourse import mybir
from concourse.bass_utils import run_bass_kernel_spmd

LAST_EXEC_NS = []
LAST_WALL_NS = []

B, L, D, H, E, TOPK = 8, 2048, 1024, 16, 64, 7
P = 128
NT = L // P   # 16 row blocks along L
ND = D // P   # 8 chunks along D
F32 = mybir.dt.float32
BF16 = mybir.dt.bfloat16
BF16_NP = ml_dtypes.bfloat16

_NC_CACHE = {}


def _get_runner(nc):
    """Cached jit runner: replicated weights, device-resident zero output
    buffers (no per-call host->device upload of them), no donation so the
    cached zeros stay valid, bf16 output fetch."""
    import jax
    import jax.numpy as jnp
    from jax.sharding import Mesh, PartitionSpec, NamedSharding
    from jax.experimental.shard_map import shard_map
    from concourse.bass2jax import (_bass_exec_p, install_neuronx_cc_hook,
                                    partition_id_tensor)
    install_neuronx_cc_hook()

    partition_name = (nc.partition_id_tensor.name
                      if nc.partition_id_tensor else None)
    in_names, out_names, out_avals = [], [], []
    for alloc in nc.m.functions[0].allocations:
        if not isinstance(alloc, mybir.MemoryLocationSet):
            continue
        name = alloc.memorylocations[0].name
        if alloc.kind == "ExternalInput":
            if name != partition_name:
                in_names.append(name)
        elif alloc.kind == "ExternalOutput":
            out_names.append(name)
            out_avals.append(jax.core.ShapedArray(
                tuple(alloc.tensor_shape), mybir.dt.np(alloc.dtype)))
    assert in_names == ["v", "W", "Sa", "Sb", "bb"], in_names
    in_names_all = in_names + out_names + (
        [partition_name] if partition_name else [])

    def _body(*args):
        operands = list(args)
        if partition_name is not None:
            operands.append(partition_id_tensor())
        outs = _bass_exec_p.bind(
            *operands,
            out_avals=tuple(out_avals),
            in_names=tuple(in_names_all),
            out_names=tuple(out_names),
            lowering_input_output_aliases=(),
            sim_require_finite=True,
            sim_require_nnan=True,
            nc=nc)
        return tuple(outs)

    devices = jax.devices()[:B]
    mesh = Mesh(np.asarray(devices), ("core",))
    SH = PartitionSpec("core")
    RE = PartitionSpec()
    # param order: v, W, Sa, Sb, bb, then zero output buffers
    in_specs = (SH, RE, SH, SH, RE) + (SH,) * len(out_names)
    out_specs = (SH,) * len(out_names)
    sharded = jax.jit(
        shard_map(_body, mesh=mesh, in_specs=in_specs,
                  out_specs=out_specs, check_rep=False),
        keep_unused=True)
    zero_sh = NamedSharding(mesh, SH)
    zeros = [
        jax.jit(lambda a=a: jnp.zeros((B * a.shape[0],) + a.shape[1:],
                                      a.dtype),
                out_shardings=zero_sh)()
        for a in out_avals
    ]
    for z in zeros:
        z.block_until_ready()
    # upload W sharded (2MB over the tunnel) and replicate device-side
    gather_W = jax.jit(
        shard_map(lambda w: jax.lax.all_gather(w, "core", axis=0,
                                               tiled=True),
                  mesh=mesh, in_specs=(SH,), out_specs=RE,
                  check_rep=False))
    v_sharding = NamedSharding(mesh, SH)
    return sharded, zeros, gather_W, v_sharding


def _bf16_to_f32(a):
    """ml_dtypes.astype is slow on large arrays; widen via integer shift."""
    return (np.asarray(a).view(np.uint16).astype(np.uint32) << 16).view(
        np.float32)


def _run(nc, v_dev, W_bf, Sa_cat, Sb_cat, bb):
    import time
    t0 = time.time()
    sharded, zeros, gather_W, _ = _NC_CACHE["runner"]
    W_dev = gather_W(W_bf)
    outs = sharded(v_dev, W_dev, Sa_cat, Sb_cat, bb, *zeros)
    out = _bf16_to_f32(outs[0]).reshape(B, L, D)
    LAST_WALL_NS.append(int((time.time() - t0) * 1e9))
    return out


def build_kernel():
    nc = bacc.Bacc()
    v_d = nc.declare_dram_parameter("v", [L, D], BF16, isOutput=False)
    W_d = nc.declare_dram_parameter("W", [D, D], BF16, isOutput=False)
    Sa_d = nc.declare_dram_parameter("Sa", [P, P], BF16, isOutput=False)
    Sb_d = nc.declare_dram_parameter("Sb", [P, P], BF16, isOutput=False)
    bb_d = nc.declare_dram_parameter("bb", [1, D], F32, isOutput=False)
    out_d = nc.declare_dram_parameter("out", [L, D], BF16, isOutput=True)

    with tile.TileContext(nc) as tc:
        with (
            tc.tile_pool(name="vbf", bufs=1) as vp,
            tc.tile_pool(name="wbf", bufs=1) as wp,
            tc.tile_pool(name="sbf", bufs=1) as sp,
            tc.tile_pool(name="agg", bufs=1) as agp,
            tc.tile_pool(name="outs", bufs=3) as otp,
            tc.tile_pool(name="psw", bufs=1, space="PSUM") as psw,
            tc.tile_pool(name="psb", bufs=2, space="PSUM") as psb,
            tc.tile_pool(name="pso", bufs=4, space="PSUM") as pso,
        ):
            Sa = sp.tile([P, P], BF16, name="Sa")
            Sb = sp.tile([P, P], BF16, name="Sb")
            nc.sync.dma_start(Sa[:], Sa_d[:, :])
            nc.sync.dma_start(Sb[:], Sb_d[:, :])

            # PE warmup (HAM clock ramp) overlapping the input DMAs
            ones = sp.tile([P, 1], F32, name="ones")
            nc.vector.memset(ones[:], 1.0)
            warm = psw.tile([1, 1], F32, tag="warm")
            nc.tensor.matmul(warm[:], ones[:], ones[:], start=True, stop=True)

            # broadcast bias [1, D] -> [128, D] via ones-outer-product matmul
            bb_row = sp.tile([1, D], F32, name="bbrow")
            nc.sync.dma_start(bb_row[:], bb_d[:, :])
            ones_r = sp.tile([1, P], F32, name="onesr")
            nc.vector.memset(ones_r[:], 1.0)
            bias = sp.tile([P, D], F32, name="bias")
            for nh in range(2):
                sl = slice(nh * 512, (nh + 1) * 512)
                ps = psb.tile([P, 512], F32)
                nc.tensor.matmul(ps[:], ones_r[:], bb_row[:, sl],
                                 start=True, stop=True)
                nc.vector.tensor_copy(bias[:, sl], ps[:])

            v_t = []
            for m in range(NT):
                t = vp.tile([P, D], BF16, tag=f"v{m}", name=f"v{m}")
                nc.sync.dma_start(t[:], v_d[m * P:(m + 1) * P, :])
                v_t.append(t)
            W_t = []
            for c in range(ND):
                t = wp.tile([P, D], BF16, tag=f"W{c}", name=f"W{c}")
                nc.sync.dma_start(t[:], W_d[c * P:(c + 1) * P, :])
                W_t.append(t)

            # banded circular aggregation: aggT[dc] = [d=128, t=2048] bf16
            # aggT[d, t] = sum_tin v[tin, d] * (Sa|Sb)[tin, t]
            agg_t = [agp.tile([P, L], BF16, tag=f"agg{c}", name=f"agg{c}")
                     for c in range(ND)]
            for mg in range(NT // 4):
                for dc in range(ND):
                    ps = psb.tile([P, 512], F32)
                    dsl = slice(dc * P, (dc + 1) * P)
                    for j in range(4):
                        m = mg * 4 + j
                        osl = slice(j * P, (j + 1) * P)
                        nc.tensor.matmul(ps[:, osl], v_t[m][:, dsl], Sa[:],
                                         start=True, stop=False)
                        nc.tensor.matmul(ps[:, osl], v_t[(m + 1) % NT][:, dsl],
                                         Sb[:], start=False, stop=True)
                    nc.vector.tensor_copy(
                        agg_t[dc][:, mg * 512:(mg + 1) * 512], ps[:])

            # out[m] = agg[:, m].T @ W + bias   -> [t=128, n=1024] bf16
            for m in range(NT):
                ot = otp.tile([P, D], BF16)
                for nh in range(2):
                    sl = slice(nh * 512, (nh + 1) * 512)
                    ps = pso.tile([P, 512], F32)
                    for kc in range(ND):
                        nc.tensor.matmul(
                            ps[:],
                            agg_t[kc][:, m * P:(m + 1) * P],
                            W_t[kc][:, sl],
                            start=(kc == 0), stop=(kc == ND - 1))
                    nc.vector.scalar_tensor_tensor(
                        ot[:, sl], ps[:], 1.0, bias[:, sl],
                        op0=mybir.AluOpType.mult, op1=mybir.AluOpType.add)
                nc.sync.dma_start(out_d[m * P:(m + 1) * P, :], ot[:])
    nc.compile()
    return nc


def _softmax(x, axis=-1):
    m = x.max(axis=axis, keepdims=True)
    e = np.exp(x - m)
    return e / e.sum(axis=axis, keepdims=True)


def host_prep(queries, keys, Wq, bq, Wk, bk, Wv, bv, Wo, bo):
    """Column sums -> top-k shifts + softmax -> band matrices; fold Wv@Wo."""
    csq = queries.sum(axis=1, dtype=np.float64)          # [B, D]
    csk = keys.sum(axis=1, dtype=np.float64)             # [B, D]
    qs = csq @ Wq.astype(np.float64) + L * bq.astype(np.float64)
    ks = csk @ Wk.astype(np.float64) + L * bk.astype(np.float64)
    mv = (qs.reshape(B, H, E) * ks.reshape(B, H, E)).sum(1) / (H * L)  # [B,E]
    idx = np.argsort(-mv.mean(0), kind="stable")[:TOPK]
    w = _softmax(mv[:, idx], axis=-1)                    # [B, TOPK]
    SaT = np.zeros((B, P, P), np.float32)
    SbT = np.zeros((B, P, P), np.float32)
    for b in range(B):
        for i, s in enumerate(idx):
            s = int(s)
            SaT[b] += np.eye(P, k=-s, dtype=np.float32) * np.float32(w[b, i])
            if s > 0:
                SbT[b] += np.eye(P, k=P - s, dtype=np.float32) * np.float32(w[b, i])
    Wf = (Wv.astype(np.float64) @ Wo.astype(np.float64)).astype(np.float32)
    bias = (bv.astype(np.float64) @ Wo.astype(np.float64) + bo).astype(np.float32)
    return SaT, SbT, Wf, bias


def kernel(**inputs):
    import jax
    f = lambda k: np.ascontiguousarray(np.asarray(inputs[k], dtype=np.float32))
    queries, keys, values = f("queries"), f("keys"), f("values")
    Wq, bq, Wk, bk = f("Wq"), f("bq"), f("Wk"), f("bk")
    Wv, bv, Wo, bo = f("Wv"), f("bv"), f("Wo"), f("bo")

    if "k1" not in _NC_CACHE:
        _NC_CACHE["k1"] = build_kernel()
    nc = _NC_CACHE["k1"]
    if "runner" not in _NC_CACHE:
        _NC_CACHE["runner"] = _get_runner(nc)
    v_sharding = _NC_CACHE["runner"][3]

    # kick off the bulk v upload first; it streams while the host computes
    # the column sums / band matrices
    v_cat = values.reshape(B * L, D).astype(BF16_NP)
    v_dev = jax.device_put(v_cat, v_sharding)

    SaT, SbT, Wf, bias = host_prep(queries, keys, Wq, bq, Wk, bk,
                                   Wv, bv, Wo, bo)
    W_bf = Wf.astype(BF16_NP)
    Sa_cat = SaT.reshape(B * P, P).astype(BF16_NP)
    Sb_cat = SbT.reshape(B * P, P).astype(BF16_NP)
    bb = np.ascontiguousarray(bias.reshape(1, D).astype(np.float32))

    return _run(nc, v_dev, W_bf, Sa_cat, Sb_cat, bb)
